# revision 2
# baseline (speedup 1.0000x reference)
"""Single-head causal self-attention on 8 trn2 NeuronCores — v3.

B=16, T=4096, D=64 fp32. Data-parallel over batch: 2 batches per core.
vs v1 baseline (175.6us):
  - exp split across ACT (real exp -> fp16) and DVE (Schraudolph int16
    bit-trick -> fp16 bit patterns, one tensor_scalar), load-balanced
    together with all PSUM->SBUF copies (qt/kt/v/out).
  - score matmul pairs use alternating 64-row halves -> concurrent
    row-group execution on the PE (2x score throughput).
  - global tile stream with lag-2 software pipeline: st(g) | exp(g-1) |
    pv(g-2), so PV never waits on a just-issued exp; yt PSUM double
    buffered across superblock boundaries.
  - superblock 0 exp forced to ACT (few-key rows see exact exp).
  - fp16 everywhere downstream of scores (PV fp16: same PE speed as
    fp8 DoubleRow on real HW, better accuracy, simpler).
"""
import os
import sys

os.environ.setdefault("MYCRO_LOCAL_CACHE", "1")
sys.path.insert(0, "/opt/trn_rl_repo")

import numpy as np

import concourse.bass as bass
import concourse.tile as tile
from concourse import bacc, mybir
from concourse.bass_utils import run_bass_kernel_spmd

F32 = mybir.dt.float32
F16 = mybir.dt.float16
U16 = mybir.dt.uint16
EXP = mybir.ActivationFunctionType.Exp

N_CORES = 8
B_LOC = 2
T = 4096
D = 64
NQ = 8
QB = 512
KB = 128
NCHUNK = T // KB
TPC = 2

L2E = 1.4426950408889634
# pt = exp(st/8 - 1.5) as f16 bits: i16 = round(1024*(L2E*(st/8-1.5) + 15 + sigma))
ACT_BIAS = -2.5
DVE_A = 1024.0 * L2E * 0.125
DVE_B = 1024.0 * (15.0 + ACT_BIAS * L2E - 0.0434)
ACT_COL, ACT_FIX = 0.833, 185.0
DVE_COL, DVE_FIX = 1.042, 125.0


def _build():
    nc = bacc.Bacc(None)

    xt_d = nc.declare_dram_parameter("xt16", [B_LOC, 128, T], F16, isOutput=False)
    wq2_d = nc.declare_dram_parameter("wq2", [128, 128], F32, isOutput=False)
    wk2_d = nc.declare_dram_parameter("wk2", [128, 128], F32, isOutput=False)
    wv_d = nc.declare_dram_parameter("wv", [128, D], F32, isOutput=False)
    id_d = nc.declare_dram_parameter("ident", [128, 128], F32, isOutput=False)
    yt1_d = nc.declare_dram_parameter("yt1", [B_LOC, NQ, D + 1, QB], F16, isOutput=True)

    bal = {"act": 0.0, "dve": 0.0}

    with tile.TileContext(nc) as tc:
        with (
            tc.tile_pool(name="consts", bufs=1) as consts,
            tc.tile_pool(name="xt", bufs=2) as xt_p,
            tc.tile_pool(name="qt", bufs=2) as qt_p,
            tc.tile_pool(name="kt", bufs=2) as kt_p,
            tc.tile_pool(name="v16", bufs=2) as v16_p,
            tc.tile_pool(name="pt", bufs=4) as pt_p,
            tc.tile_pool(name="scratch", bufs=2) as scratch_p,
            tc.tile_pool(name="stps", bufs=2, space="PSUM") as st_ps,
            tc.tile_pool(name="ytps", bufs=2, space="PSUM") as yt_ps,
            tc.tile_pool(name="prps", bufs=2, space="PSUM") as pr_ps,
        ):
            def bal_add(eng, cols):
                if eng == "act":
                    bal["act"] += ACT_COL * cols + ACT_FIX
                else:
                    bal["dve"] += DVE_COL * cols + DVE_FIX

            def bal_pick(cols):
                ca = bal["act"] + ACT_COL * cols + ACT_FIX
                cd = bal["dve"] + DVE_COL * cols + DVE_FIX
                return "act" if ca <= cd else "dve"

            def bal_copy(out, in_, cols):
                e = bal_pick(cols)
                if e == "act":
                    nc.scalar.copy(out=out, in_=in_)
                else:
                    nc.vector.tensor_copy(out=out, in_=in_)
                bal_add(e, cols)

            # ---- constants ----
            ident = consts.tile([128, 128], F16, tag="ident")
            nc.gpsimd.dma_start(out=ident, in_=id_d[:, :])
            wq2 = consts.tile([128, 128], F16, tag="wq2")
            nc.gpsimd.dma_start(out=wq2, in_=wq2_d[:, :])
            wk2 = consts.tile([128, 128], F16, tag="wk2")
            nc.gpsimd.dma_start(out=wk2, in_=wk2_d[:, :])
            wv = consts.tile([128, D], F16, tag="wv")
            nc.gpsimd.dma_start(out=wv, in_=wv_d[:, :])

            cbias = consts.tile([128, 1], F32, tag="cbias")
            nc.vector.memset(cbias, ACT_BIAS)

            # ---- warmups: ACT table load + PE HAM ramp ----
            wsc = scratch_p.tile([128, 128], F32, tag="wexp")
            nc.scalar.activation(out=wsc, in_=wq2, func=EXP, scale=0.01)
            for _ in range(8):
                wps = pr_ps.tile([128, 128], F32, tag="prj", name="wps")
                nc.tensor.matmul(out=wps, lhsT=ident, rhs=ident, start=True, stop=True)

            state = {}

            def make_prologue(b):
                xt = xt_p.tile([128, T], F16, tag="xt", name="xt")
                for dj in range(NQ):
                    nc.sync.dma_start(
                        out=xt[:, QB * dj : QB * (dj + 1)],
                        in_=xt_d[b, :, QB * dj : QB * (dj + 1)],
                    )
                qt = qt_p.tile([128, T], F16, tag="qt", name="qt")
                kt = kt_p.tile([128, T], F16, tag="kt", name="kt")
                v16 = v16_p.tile([128, NCHUNK, D + 1], F16, tag="v16", name="v16")
                nc.gpsimd.memset(v16[:, :, D : D + 1], 1.0)
                state[b] = (qt, kt, v16)

                def proj_qk(j):
                    pq = pr_ps.tile([128, QB], F32, tag="prj", name="pq")
                    hq = 64 * (j % 2)
                    nc.tensor.matmul(out=pq, lhsT=wq2[hq : hq + 64, :], rhs=xt[hq : hq + 64, QB * j : QB * (j + 1)], start=True, stop=True)
                    bal_copy(qt[:, QB * j : QB * (j + 1)], pq, QB)
                    pk = pr_ps.tile([128, QB], F32, tag="prj", name="pk")
                    nc.tensor.matmul(out=pk, lhsT=wk2[64 - hq : 128 - hq, :], rhs=xt[64 - hq : 128 - hq, QB * j : QB * (j + 1)], start=True, stop=True)
                    bal_copy(kt[:, QB * j : QB * (j + 1)], pk, QB)

                def proj_v(g):
                    pvpa = pr_ps.tile([128, QB // 2], F32, tag="prj", name="pvpa")
                    pvpb = pr_ps.tile([128, QB // 2], F32, tag="prj", name="pvpb")
                    for k in range(8):
                        t = 8 * g + k
                        h = 64 * (k % 2)
                        dst = pvpa if k % 2 == 0 else pvpb
                        nc.tensor.matmul(
                            out=dst[:, D * (k // 2) : D * (k // 2 + 1)],
                            lhsT=xt[h : h + 64, 128 * t : 128 * (t + 1)],
                            rhs=wv[h : h + 64, :],
                            start=True,
                            stop=True,
                        )
                    bal_copy(
                        v16[:, 8 * g : 8 * (g + 1) : 2, 0:D],
                        pvpa.rearrange("p (k c) -> p k c", c=D),
                        QB // 2,
                    )
                    bal_copy(
                        v16[:, 8 * g + 1 : 8 * (g + 1) : 2, 0:D],
                        pvpb.rearrange("p (k c) -> p k c", c=D),
                        QB // 2,
                    )

                return proj_qk, proj_v

            # ---- global tile stream with lag-2 pipeline ----
            # Each element: (b, m, ti) with per-superblock chunk layout.
            pending = []  # emitted st tiles awaiting exp: (tileinfo)
            expd = []     # exp'd tiles awaiting PV

            def emit_st_pair(tinfo):
                b, m, ti, st_t, yt1 = tinfo
                qt, kt, v16 = state[b]
                for slot in range(TPC):
                    c = TPC * ti + slot
                    j = c - 4 * m
                    qoff = 128 * j if j > 0 else 0
                    half = 64 * (c % 2)
                    nc.tensor.matmul(
                        out=st_t[:, QB * slot + qoff : QB * (slot + 1)],
                        lhsT=kt[half : half + 64, KB * c : KB * (c + 1)],
                        rhs=qt[half : half + 64, QB * m + qoff : QB * (m + 1)],
                        start=True,
                        stop=True,
                    )

            def emit_exp_tile(tinfo):
                b, m, ti, st_t, yt1 = tinfo
                pt = pt_p.tile([128, QB * TPC], F16, tag="pt", name="pt")
                c0 = TPC * ti
                diag = c0 + TPC - 4 * m > 0
                force_act = m == 0

                def one(in_ap, out_ap, cols):
                    eng = "act" if force_act else bal_pick(cols)
                    if eng == "act":
                        nc.scalar.activation(out=out_ap, in_=in_ap, func=EXP, bias=cbias, scale=0.125)
                    else:
                        nc.vector.tensor_scalar(
                            out=out_ap.bitcast(U16),
                            in0=in_ap,
                            scalar1=DVE_A,
                            scalar2=DVE_B,
                            op0=mybir.AluOpType.mult,
                            op1=mybir.AluOpType.add,
                        )
                    bal_add(eng, cols)

                if not diag:
                    one(st_t, pt, QB * TPC)
                else:
                    for slot in range(TPC):
                        c = c0 + slot
                        j = c - 4 * m
                        qoff = 128 * j if j > 0 else 0
                        one(
                            st_t[:, QB * slot + qoff : QB * (slot + 1)],
                            pt[:, QB * slot + qoff : QB * (slot + 1)],
                            QB - qoff,
                        )
                for slot in range(TPC):
                    c = c0 + slot
                    j = c - 4 * m
                    if j >= 0:
                        sub = pt[:, QB * slot + 128 * j : QB * slot + 128 * (j + 1)]
                        nc.gpsimd.affine_select(
                            out=sub,
                            in_=sub,
                            compare_op=mybir.AluOpType.is_ge,
                            fill=0.0,
                            base=0,
                            pattern=[[1, 128]],
                            channel_multiplier=-1,
                        )
                return pt

            def emit_pv(tinfo, pt):
                b, m, ti, st_t, yt1 = tinfo
                qt, kt, v16 = state[b]
                nch = 4 * m + 4
                for slot in range(TPC):
                    c = TPC * ti + slot
                    j = c - 4 * m
                    qoff = 128 * j if j > 0 else 0
                    nc.tensor.matmul(
                        out=yt1[0 : D + 1, qoff:QB],
                        lhsT=v16[:, c, :],
                        rhs=pt[:, QB * slot + qoff : QB * (slot + 1)],
                        start=(c == 0),
                        stop=(c == nch - 1),
                        skip_group_check=True,
                    )
                if TPC * ti + TPC == nch:
                    ytsb = scratch_p.tile([D + 1, QB], F16, tag="ytsb", name="ytsb")
                    bal_copy(ytsb, yt1[0 : D + 1, :], QB)
                    nc.sync.dma_start(out=yt1_d[b, m, :, :], in_=ytsb)

            def drain(n):
                while len(expd) > n:
                    ti2, pt2 = expd.pop(0)
                    emit_pv(ti2, pt2)

            def step(b, m, ti, yt1):
                st_t = st_ps.tile([128, QB * TPC], F32, tag="st", name="st_t")
                tinfo = (b, m, ti, st_t, yt1)
                emit_st_pair(tinfo)
                pending.append(tinfo)
                lag = 0 if os.environ.get("KV3_NO_LAG") == "1" else 1
                while len(pending) > lag:
                    t2 = pending.pop(0)
                    expd.append((t2, emit_exp_tile(t2)))
                drain(lag)

            def superblock(b, m):
                yt1 = yt_ps.tile([128, QB], F32, tag="yt1")
                for ti in range((4 * m + 4) // TPC):
                    step(b, m, ti, yt1)

            def finish():
                while pending:
                    t2 = pending.pop(0)
                    expd.append((t2, emit_exp_tile(t2)))
                drain(0)

            pq0, pv0 = make_prologue(0)
            if os.environ.get("KV3_TRUNC") == "1":
                for j in range(NQ):
                    pq0(j)
                for g in range(4):
                    pv0(g)
                superblock(0, 0)
                superblock(0, 1)
                finish()
            else:
                pq0(0); pv0(0)
                superblock(0, 0)
                pq0(1); superblock(0, 1)
                pq0(2); pv0(1); superblock(0, 2)
                pq0(3); superblock(0, 3)
                pq0(4); pv0(2); superblock(0, 4)
                pq1, pv1 = make_prologue(1)
                pq0(5); pv0(3); superblock(0, 5)
                pq0(6); pq1(0); pv1(0); superblock(0, 6)
                pq0(7); pq1(1); pq1(2); pv1(1); superblock(0, 7)
                pq1(3); pq1(4); pv1(2); superblock(1, 0)
                pq1(5); pq1(6); pv1(3); superblock(1, 1)
                pq1(7); superblock(1, 2)
                for m in range(3, NQ):
                    superblock(1, m)
                finish()

    nc.finalize()
    return nc


def build():
    return _build()


_NC = None


def _get_nc():
    global _NC
    if _NC is None:
        _NC = _build()
    return _NC


def _run(x, Wk, Wq, Wv, trace=False):
    x = np.ascontiguousarray(np.asarray(x, dtype=np.float32))
    Wk = np.asarray(Wk, dtype=np.float32)
    Wq = np.asarray(Wq, dtype=np.float32)
    Wv = np.asarray(Wv, dtype=np.float32)
    B = x.shape[0]
    assert B == N_CORES * B_LOC and x.shape[1] == T and x.shape[2] == D

    wq2 = np.concatenate([Wq.T, Wq.T], axis=1)
    wq2 = np.ascontiguousarray(np.concatenate([wq2, wq2], axis=0))
    wk2 = np.concatenate([Wk.T, Wk.T], axis=1)
    wk2 = np.ascontiguousarray(np.concatenate([wk2, wk2], axis=0))
    wv = np.ascontiguousarray(np.concatenate([Wv.T, Wv.T], axis=0))
    ident = np.eye(128, dtype=np.float32)

    xt16 = x.astype(np.float16).transpose(0, 2, 1)
    xt16 = np.ascontiguousarray(np.concatenate([xt16, xt16], axis=1))
    in_maps = []
    for c in range(N_CORES):
        in_maps.append(
            {
                "xt16": np.ascontiguousarray(xt16[B_LOC * c : B_LOC * (c + 1)]),
                "wq2": wq2,
                "wk2": wk2,
                "wv": wv,
                "ident": ident,
            }
        )

    nc = _get_nc()
    res = run_bass_kernel_spmd(nc, in_maps, core_ids=list(range(N_CORES)), trace=trace)

    y = np.empty((B, T, D), dtype=np.float32)
    for c in range(N_CORES):
        yt1 = np.asarray(res.results[c]["yt1"], dtype=np.float32)
        num = yt1[:, :, :D, :]
        den = yt1[:, :, D : D + 1, :]
        yb = (num / den).transpose(0, 1, 3, 2).reshape(B_LOC, T, D)
        y[B_LOC * c : B_LOC * (c + 1)] = yb
    return y, res


def kernel(x, Wk, Wq, Wv):
    y, _ = _run(x, Wk, Wq, Wv, trace=False)
    return y
